# revision 1
# baseline (speedup 1.0000x reference)
"""Trainium2 Bass kernel for nn_CausalAttention_76304388981436.

Full-input contract: kernel(**inputs) -> [2, 2048, 512] f32.

Sharding (8 cores, single SPMD program): core c = (batch b=c//4, head-pair
hp=c%4).  Each core computes attention for its 2 heads over the full 2048
sequence of its batch, producing per-head UNNORMALIZED projected numerators
outh[h] = (sum_j exp(S-11) vT)^T @ Wo[h-rows]  [2048, 512] f16 plus the
softmax denominators dens [8, 512] f16; the host divides per head, sums the
4 head-pair partials per batch and adds bo.

Device-side math per core (transposed-attention layout, heads packed at
partitions 0-63 / 64-127 of the PE array):
  qT[128i, 2048n] = wq2^T x^T   (wq2 pre-scaled by 1/8 on host)
  kT[128i, 2048n] = wk2^T x^T
  v [2048n, 2x65] = x wv2       (col 64 of each head block memset to 1.0 ->
                                 PV matmul also produces softmax denominator)
  P' tiles [128q, 512s] = qT_h^T rel_embT_rev  (rel table pre-reversed on
      host; only the live s-chunks per q-chunk are computed/stored)
  PR dram per (h, qc) [512q, 2048s] fp16: bias[j,q] = PR[q, j-q+1023]
  skew read: ONE transposing DMA per (h, qc) with source AP
      [[2047, 512], [1, 128*njt]] (contiguous 2-3KB runs) -> SBUF
      [128j, njt, 512q] covering every in-band j-tile of the q-chunk.
  S^T tile [128j, 512q] = kT_h^T qT_h; bias added via a SECOND accumulating
      matmul ident_f16^T @ bias_tile (in-band) or the clamp-value matmul
      (out-of-band), all into the same PSUM accumulation group.
  expS = exp(S^T - 11)  one ACT op per [128, 1024] PSUM bank-pair
  outT_h [65, 512q] += v_ext_h^T expS   (row 64 = denominator)
  outh[h] [2048q, 512] = numT_h^T wo2_h  (unnormalized, fp16 out)
"""
import numpy as np
import ml_dtypes

import concourse.bass as bass
import concourse.mybir as mybir
import concourse.tile as tile
from concourse.bass_utils import run_bass_kernel_spmd

F32 = mybir.dt.float32
F32R = mybir.dt.float32r
F16 = mybir.dt.float16
BF16 = mybir.dt.bfloat16
AF = mybir.ActivationFunctionType

N = 2048          # sequence length
D = 512           # model dim
HD = 64           # head dim
NQC = 4           # q-chunks of 512
NJT = 16          # j-tiles of 128
W = 2048          # PR row width
SHIFT = -11.0     # exp(logit + SHIFT): keeps num/den in fp16 range

# per q-chunk: (jt_min, njt) of in-band j-tiles (-512 < A < 1152,
# A = 512*(qc+1) - 128*jt)
IN_BAND = {0: (0, 8), 1: (0, 12), 2: (4, 12), 3: (8, 8)}
# per q-chunk: live 512-wide s-chunks of PR (others never read)
LIVE_CI = {0: (1, 2, 3), 1: (0, 1, 2, 3), 2: (0, 1, 2, 3), 3: (0, 1, 2)}


def _split_multiwaits(nc):
    """This walrus build rejects >1 sync wait per instruction; split extra
    waits onto single-wait NoOps on the same engine just before it."""
    for func in nc.m.functions:
        for block in func.blocks:
            new_instrs = []
            for inst in block.instructions:
                si = inst.sync_info
                if si is not None and si.on_wait and len(si.on_wait) > 1:
                    waits = list(si.on_wait)
                    for w in waits[:-1]:
                        new_instrs.append(mybir.InstNoOp(
                            name=nc.get_next_instruction_name(),
                            engine=inst.engine,
                            bass_nofuse=True,
                            sync_info=mybir.SyncInfo(on_wait=[w], on_update=[]),
                        ))
                    si.on_wait = waits[-1:]
                new_instrs.append(inst)
            block.instructions = new_instrs


def _r(ap):
    return ap.bitcast(F32R)


def build_kernel():
    nc = bass.Bass("TRN2")
    xT = nc.dram_tensor("xT", [D, N], F32, kind="ExternalInput")
    wq2 = nc.dram_tensor("wq2", [D, 128], F32, kind="ExternalInput")
    wk2 = nc.dram_tensor("wk2", [D, 128], F32, kind="ExternalInput")
    wv2 = nc.dram_tensor("wv2", [D, 128], F32, kind="ExternalInput")
    wo2 = nc.dram_tensor("wo2", [128, D], F32, kind="ExternalInput")
    relT = nc.dram_tensor("relT", [128, W], BF16, kind="ExternalInput")
    relbc = nc.dram_tensor("relbc", [128, 256], BF16, kind="ExternalInput")
    ones2 = nc.dram_tensor("ones2", [128, 2, 1], F32, kind="ExternalInput")
    ident = nc.dram_tensor("ident", [128, 128], F16, kind="ExternalInput")
    outh = nc.dram_tensor("outh", [2, N, D], F16, kind="ExternalOutput")
    dens = nc.dram_tensor("dens", [8, 512], F16, kind="ExternalOutput")

    with tile.TileContext(nc) as tc:
        _build_body(nc, tc, xT, wq2, wk2, wv2, wo2, relT, relbc, ones2,
                    ident, outh, dens)
    _split_multiwaits(nc)
    return nc


def _build_body(nc, tc, xT, wq2, wk2, wv2, wo2, relT, relbc, ones2, ident,
                outh, dens):
    from contextlib import ExitStack
    ctx = ExitStack()
    consts = ctx.enter_context(tc.tile_pool(name="consts", bufs=1))
    qkv = ctx.enter_context(tc.tile_pool(name="qkv", bufs=1))
    pc = ctx.enter_context(tc.tile_pool(name="pc", bufs=3))
    skew = ctx.enter_context(tc.tile_pool(name="skew", bufs=2))
    exps = ctx.enter_context(tc.tile_pool(name="exps", bufs=12))
    outc = ctx.enter_context(tc.tile_pool(name="outc", bufs=3))
    dent = ctx.enter_context(tc.tile_pool(name="dent", bufs=4))
    psa = ctx.enter_context(tc.tile_pool(name="psa", bufs=1, space="PSUM"))
    pos = ctx.enter_context(tc.tile_pool(name="pos", bufs=1, space="PSUM"))
    pp = ctx.enter_context(tc.tile_pool(name="pp", bufs=1, space="PSUM"))
    pdram = ctx.enter_context(tc.tile_pool(name="pdram", bufs=1, space="DRAM"))

    # ---- load constants / inputs ----
    sxT = [consts.tile([128, N], F32, name=f"xT{i}", tag=f"xT{i}") for i in range(4)]
    for i in range(4):
        nc.sync.dma_start(out=_r(sxT[i][:]), in_=_r(xT[i * 128:(i + 1) * 128, :]))
    swq = [consts.tile([128, 128], F32, name=f"wq{i}", tag=f"wq{i}") for i in range(4)]
    swk = [consts.tile([128, 128], F32, name=f"wk{i}", tag=f"wk{i}") for i in range(4)]
    swv = [consts.tile([128, 128], F32, name=f"wv{i}", tag=f"wv{i}") for i in range(4)]
    for i in range(4):
        nc.sync.dma_start(out=_r(swq[i][:]), in_=_r(wq2[i * 128:(i + 1) * 128, :]))
        nc.sync.dma_start(out=_r(swk[i][:]), in_=_r(wk2[i * 128:(i + 1) * 128, :]))
        nc.sync.dma_start(out=_r(swv[i][:]), in_=_r(wv2[i * 128:(i + 1) * 128, :]))
    swo = consts.tile([128, D], F32, name="wo", tag="wo")
    nc.sync.dma_start(out=_r(swo[:]), in_=_r(wo2[:, :]))
    srelT = consts.tile([128, W], BF16, name="relT", tag="relT")
    nc.sync.dma_start(out=srelT[:], in_=relT[:, :])
    srelbc = consts.tile([128, 256], BF16, name="relbc", tag="relbc")
    nc.sync.dma_start(out=srelbc[:], in_=relbc[:, :])
    sones = consts.tile([128, 2, 1], F32, name="ones2", tag="ones2")
    nc.sync.dma_start(out=_r(sones[:]), in_=_r(ones2[:, :, :]))
    sident = consts.tile([128, 128], F16, name="ident", tag="ident")
    nc.sync.dma_start(out=sident[:], in_=ident[:, :])
    sbias = consts.tile([128, 1], F32, name="sbias", tag="sbias")
    nc.gpsimd.memset(sbias[:], SHIFT)

    # ---- projections ----
    qT = qkv.tile([128, N], BF16, name="qT", tag="qT")
    kT = qkv.tile([128, N], BF16, name="kT", tag="kT")
    for nchunk in range(NQC):
        ns = slice(nchunk * 512, nchunk * 512 + 512)
        for di, (dst, w) in enumerate(((qT, swq), (kT, swk))):
            ps = pp.tile([128, 1024], F32, name="pp", tag="pp")
            half = ps[:, di * 512:di * 512 + 512]
            for c in range(4):
                nc.tensor.matmul(half, _r(w[c][:]), _r(sxT[c][:, ns]),
                                 start=(c == 0), stop=(c == 3))
            nc.vector.tensor_copy(out=dst[:, ns], in_=half)
    # v in natural layout with ones column per head block
    vt = [qkv.tile([128, 2, 65], BF16, name=f"v{t}", tag=f"v{t}") for t in range(NJT)]
    for t in range(NJT):
        nst = slice(t * 128, t * 128 + 128)
        ps = pp.tile([128, 1024], F32, name="pp", tag="pp")
        half = ps[:, (t % 2) * 512:(t % 2) * 512 + 128]
        for c in range(4):
            nc.tensor.matmul(half, _r(sxT[c][:, nst]), _r(swv[c][:]),
                             start=(c == 0), stop=(c == 3))
        nc.vector.tensor_copy(out=vt[t][:, :, 64:65], in_=sones[:])
        nc.vector.tensor_copy(out=vt[t][:, :, 0:64],
                              in_=half.rearrange("p (h d) -> p h d", h=2))

    # ---- per-(h, qc) PR dram tensors ----
    prd = {(h, qc): pdram.tile([512, W], F16, name=f"pr{h}_{qc}",
                               tag=f"pr{h}_{qc}")
           for h in range(2) for qc in range(NQC)}

    def p_units_for(qc):
        """Fine-grained closures producing P'(qc): one per (qt, ci) matmul+
        copy step, plus one per qt for the PR write DMAs."""
        if qc >= NQC:
            return []
        units = []
        cis = LIVE_CI[qc]
        state = {}
        for qt_local in range(4):
            qt = 4 * qc + qt_local

            def start_qt(qt=qt):
                state[qt] = pc.tile([128, 2, W], F16, name="pct", tag="pct")

            units.append(start_qt)
            for ci in cis:
                def do_ci(qt=qt, ci=ci):
                    qs = slice(qt * 128, qt * 128 + 128)
                    cs = slice(ci * 512, ci * 512 + 512)
                    ps = pp.tile([128, 1024], F32, name="pp", tag="pp")
                    for h in range(2):
                        hs = slice(h * 64, h * 64 + 64)
                        nc.tensor.matmul(ps[:, h * 512:h * 512 + 512],
                                         qT[hs, qs], srelT[hs, cs],
                                         start=True, stop=True,
                                         tile_position=(h * 64, 0))
                    nc.vector.tensor_copy(
                        out=state[qt][:, :, cs],
                        in_=ps[:].rearrange("p (h s) -> p h s", h=2))

                units.append(do_ci)

            def write_qt(qt=qt, qt_local=qt_local):
                lo, hi = cis[0] * 512, cis[-1] * 512 + 512
                rows = slice(qt_local * 128, qt_local * 128 + 128)
                pct = state[qt]
                for h in range(2):
                    nc.gpsimd.dma_start(out=prd[(h, qc)][rows, lo:hi],
                                        in_=pct[:, h, lo:hi])

            units.append(write_qt)
        return units

    def emit_skew_read(qc):
        """One transposing DMA per head covering all in-band j-tiles."""
        jt_min, njt = IN_BAND[qc]
        tiles = {}
        for h in range(2):
            skt = skew.tile([128, 12, 512], F16, name="skt", tag=f"skt{h}")
            t = prd[(h, qc)]
            src = bass.AP(tensor=t.tensor,
                          offset=t.offset + 128 * jt_min - 512 * qc + 1023,
                          ap=[[2047, 512], [1, 128 * njt]])
            nc.sync.dma_start(out=skt[:, 0:njt, :], in_=src, transpose=True)
            tiles[h] = skt
        return tiles

    # ---- attention ----
    # ah holds both heads' unnormalized numerators: rows 0-63 h0, 64-127 h1
    ah = qkv.tile([128, N], F32, name="ah", tag="ah")

    def emit_attn(qc, skt, p_units):
        jt_min, njt = IN_BAND[qc]
        qs = slice(qc * 512, qc * 512 + 512)
        pot = [pos.tile([65, 512], F32, name="po", tag=f"po{h}")
               for h in range(2)]
        ets = {0: [None] * 8, 1: [None] * 8}
        pu = iter(p_units)

        def emit_pv(p):
            for h in range(2):
                for idx in range(2):
                    jt = 2 * p + idx
                    nc.tensor.matmul(pot[h][:], vt[jt][:, h, :],
                                     ets[h][p][:, idx * 512:idx * 512 + 512],
                                     start=(jt == 0), stop=(jt == NJT - 1))

        for p in range(8):
            pst = {}
            for h in range(2):
                pst[h] = psa.tile([128, 1024], F32, name="psa", tag=f"psa{h}")
            # S matmuls: heads adjacent -> concurrent PE row groups
            for idx in range(2):
                jt = 2 * p + idx
                js = slice(jt * 128, jt * 128 + 128)
                for h in range(2):
                    hs = slice(h * 64, h * 64 + 64)
                    nc.tensor.matmul(pst[h][:, idx * 512:idx * 512 + 512],
                                     kT[hs, js], qT[hs, qs],
                                     start=True, stop=False,
                                     tile_position=(h * 64, 0))
            # bias adds
            for idx in range(2):
                jt = 2 * p + idx
                A = qc * 512 + 512 - 128 * jt
                in_band = not (A <= -512 or A >= 1152)
                for h in range(2):
                    hs = slice(h * 64, h * 64 + 64)
                    half = pst[h][:, idx * 512:idx * 512 + 512]
                    if in_band:
                        nc.tensor.matmul(half, sident[:],
                                         skt[h][:, jt - jt_min, :],
                                         start=False, stop=True)
                    else:
                        bc = 0 if A <= -512 else 128
                        nc.tensor.matmul(half, srelbc[hs, bc:bc + 128],
                                         qT[hs, qs], start=False, stop=True,
                                         tile_position=(h * 64, 0))
            for h in range(2):
                et = exps.tile([128, 1024], BF16, name="expS", tag="expS")
                nc.scalar.activation(out=et[:], in_=pst[h][:], func=AF.Exp,
                                     bias=sbias[:])
                ets[h][p] = et
            if p > 0:
                emit_pv(p - 1)
            # interleave P'(qc+1) production
            for _ in range(3):
                u = next(pu, None)
                if u is not None:
                    u()
        emit_pv(7)
        for u in pu:
            u()
        # numerators -> ah (f32), denominators -> dram (f16)
        for h in range(2):
            hs = slice(h * 64, h * 64 + 64)
            nc.vector.tensor_copy(out=_r(ah[hs, qs]), in_=_r(pot[h][0:64, :]))
            dt = dent.tile([1, 512], F16, name="den", tag="den")
            nc.vector.tensor_copy(out=dt[:], in_=pot[h][64:65, :])
            nc.gpsimd.dma_start(out=dens[h * 4 + qc:h * 4 + qc + 1, :],
                                in_=dt[:])

    # ---- pipeline: P'(0) up front, then attn(qc) || P'(qc+1) ----
    for u in p_units_for(0):
        u()
    for qc in range(NQC):
        skt = emit_skew_read(qc)
        emit_attn(qc, skt, p_units_for(qc + 1))

    # ---- output projection (per head, unnormalized) ----
    for qt in range(NJT):
        qs = slice(qt * 128, qt * 128 + 128)
        ps = psa.tile([128, 1024], F32, name="psa", tag=f"psa{qt % 2}")
        for h in range(2):
            hs = slice(h * 64, h * 64 + 64)
            nc.tensor.matmul(ps[:, h * 512:h * 512 + 512],
                             _r(ah[hs, qs]), _r(swo[hs, :]),
                             start=True, stop=True,
                             tile_position=(h * 64, 0))
        ot = outc.tile([128, 1024], F16, name="oc", tag="oc")
        nc.vector.tensor_copy(out=ot[:], in_=ps[:])
        for h in range(2):
            nc.gpsimd.dma_start(out=outh[h, qs, :],
                                in_=ot[:, h * 512:h * 512 + 512])
    ctx.close()


_NC_CACHE = [None]


def _get_nc():
    if _NC_CACHE[0] is None:
        _NC_CACHE[0] = build_kernel()
    return _NC_CACHE[0]


def make_in_maps(x, Wq, Wkv, Wo, bo, rel_emb):
    xT = [np.ascontiguousarray(x[b].T).astype(np.float32) for b in range(2)]
    cols = np.arange(W)
    idx = np.clip(1535 - cols, 0, 1024)
    relT = np.empty((128, W), np.float32)
    relT[0:64] = rel_emb[idx].T
    relT[64:128] = relT[0:64]
    relT = relT.astype(ml_dtypes.bfloat16)          # reversed rel table
    relbc = np.empty((128, 256), np.float32)
    relbc[0:64, 0:128] = rel_emb[0][:, None]       # clamp-low value
    relbc[0:64, 128:256] = rel_emb[1024][:, None]  # clamp-high value
    relbc[64:128] = relbc[0:64]
    relbc = relbc.astype(ml_dtypes.bfloat16)
    ident = np.eye(128, dtype=np.float16)
    in_maps = []
    for c in range(8):
        b, hp = c // 4, c % 4
        cs = slice(hp * 128, hp * 128 + 128)
        in_maps.append({
            "xT": xT[b],
            "wq2": np.ascontiguousarray(Wq[:, cs] / 8.0).astype(np.float32),
            "wk2": np.ascontiguousarray(Wkv[:, :512][:, cs]).astype(np.float32),
            "wv2": np.ascontiguousarray(Wkv[:, 512:][:, cs]).astype(np.float32),
            "wo2": np.ascontiguousarray(Wo[cs, :]).astype(np.float32),
            "relT": relT,
            "relbc": relbc,
            "ones2": np.ones((128, 2, 1), np.float32),
            "ident": ident,
        })
    return in_maps


def run(x, Wq, Wkv, Wo, bo, rel_emb, trace=False, trace_cores=None):
    nc = _get_nc()
    in_maps = make_in_maps(x, Wq, Wkv, Wo, bo, rel_emb)
    res = run_bass_kernel_spmd(nc, in_maps, core_ids=list(range(8)),
                               trace=trace, trace_cores=trace_cores)
    out = np.zeros((2, N, D), np.float32)
    for c in range(8):
        b = c // 4
        num = np.asarray(res.results[c]["outh"], np.float32)   # [2, N, D]
        den = np.asarray(res.results[c]["dens"], np.float32)   # [8, 512]
        for h in range(2):
            out[b] += num[h] / den[h * 4:(h + 1) * 4].reshape(N)[:, None]
    out += np.asarray(bo, np.float32)[None, None, :]
    return out, res


def kernel(x, Wq, Wkv, Wo, bo, rel_emb):
    out, _ = run(np.asarray(x), np.asarray(Wq), np.asarray(Wkv),
                 np.asarray(Wo), np.asarray(bo), np.asarray(rel_emb))
    return out



# revision 5
# speedup vs baseline: 1.3425x; 1.3425x over previous
"""Trainium2 Bass kernel for nn_CausalAttention_76304388981436.

Full-input contract: kernel(**inputs) -> [2, 2048, 512] f32.

Sharding (8 cores, single SPMD program): core c = (batch b=c//4, head-pair
hp=c%4).  Each core computes attention for its 2 heads over the full 2048
sequence of its batch, producing per-head UNNORMALIZED projected numerators
outh[h] = (sum_j exp(S-11) vT)^T @ Wo[h-rows]  [2048, 512] f16 plus the
softmax denominators dens2 [4, 2, 512] f16; the host divides per head, sums
the 4 head-pair partials per batch and adds bo.

Device-side math per core (transposed-attention layout, heads packed at
partitions 0-63 / 64-127 of the PE array):
  qT[128i, 2048n] = wq2^T x^T   (wq2 pre-scaled by 1/8 on host; all inputs
                                 bf16 on host to halve DMA + run PE 1cyc/row)
  kT[128i, 2048n] = wk2^T x^T
  vta[128n, 16t, 2h, 65] = x wv2 (+ ones col 64 -> PV also produces denom)
  P' tiles [128q, 512s] = qT_h^T rel_embT_rev  (rel table pre-reversed on
      host; only the live s-chunks per q-chunk are computed/stored)
  PR dram per qc [2h, 512q, 2048s] fp16: bias[j,q] = PR[h, q, j-q+1023]
  skew read: ONE transposing DMA per (h, qc) with source AP
      [[2047, 512], [1, 128*njt]] (contiguous 2-3KB runs) -> SBUF
      [128j, njt, 512q]; h0 on the SP HWDGE queue, h1 on the ACT HWDGE
      queue so the two transfers overlap.
  S^T tile [128j, 1024(h0q|h1q)] = kT_h^T qT_h (both heads tile_position-
      paired); bias added via accumulating ident_f16 matmul (in-band) or
      clamp-value matmul (out-of-band) into the same PSUM group.
  expS = exp(S^T - 11)  one ACT op per [128, 1024] jt tile
  outT_h [65, 512q] += vta_h^T expS   (row 64 = denominator)
  outh[h] [2048q, 512] = numT_h^T wo2_h  (unnormalized, fp16 out)

Scheduling: per q-chunk the OUT-OF-BAND j-tiles run first so the skew DMA
(issued mid-previous-iteration right after the PR writes) overlaps compute.
P'(qc+1) units are front-loaded as PE filler inside attn(qc); outproj(qc-1)
trails at the iteration end.  Prologue pipelines x-chunk DMAs with q-proj,
P'(0), k/v-proj so the PE never sits idle waiting for input DMAs.
"""
import numpy as np
import ml_dtypes

import concourse.bass as bass
import concourse.mybir as mybir
import concourse.tile as tile
from concourse.bass_utils import run_bass_kernel_spmd

F32 = mybir.dt.float32
F32R = mybir.dt.float32r
F16 = mybir.dt.float16
BF16 = mybir.dt.bfloat16
AF = mybir.ActivationFunctionType

N = 2048          # sequence length
D = 512           # model dim
HD = 64           # head dim
NQC = 4           # q-chunks of 512
NJT = 16          # j-tiles of 128
W = 2048          # PR row width
SHIFT = -11.0     # exp(logit + SHIFT): keeps num/den in fp16 range

# per q-chunk: (jt_min, njt) of in-band j-tiles (-512 < A < 1152,
# A = 512*(qc+1) - 128*jt)
IN_BAND = {0: (0, 8), 1: (0, 12), 2: (4, 12), 3: (8, 8)}
# per q-chunk: live 512-wide s-chunks of PR (others never read)
LIVE_CI = {0: (1, 2, 3), 1: (0, 1, 2, 3), 2: (0, 1, 2, 3), 3: (0, 1, 2)}


def _split_multiwaits(nc):
    """This walrus build rejects >1 sync wait per instruction; split extra
    waits onto single-wait NoOps on the same engine just before it."""
    for func in nc.m.functions:
        for block in func.blocks:
            new_instrs = []
            for inst in block.instructions:
                si = inst.sync_info
                if si is not None and si.on_wait and len(si.on_wait) > 1:
                    waits = list(si.on_wait)
                    for w in waits[:-1]:
                        new_instrs.append(mybir.InstNoOp(
                            name=nc.get_next_instruction_name(),
                            engine=inst.engine,
                            bass_nofuse=True,
                            sync_info=mybir.SyncInfo(on_wait=[w], on_update=[]),
                        ))
                    si.on_wait = waits[-1:]
                new_instrs.append(inst)
            block.instructions = new_instrs


def _r(ap):
    return ap.bitcast(F32R)


def build_kernel():
    nc = bass.Bass("TRN2")
    xT = nc.dram_tensor("xT", [D, N], BF16, kind="ExternalInput")
    wqkv = nc.dram_tensor("wqkv", [4, 128, 3, 128], BF16, kind="ExternalInput")
    wo2 = nc.dram_tensor("wo2", [128, D], F32, kind="ExternalInput")
    relT = nc.dram_tensor("relT", [128, W], BF16, kind="ExternalInput")
    relbc = nc.dram_tensor("relbc", [128, 256], BF16, kind="ExternalInput")
    ident = nc.dram_tensor("ident", [128, 128], F16, kind="ExternalInput")
    outh = nc.dram_tensor("outh", [2, N, D], F16, kind="ExternalOutput")
    dens2 = nc.dram_tensor("dens2", [4, 2, 512], F16, kind="ExternalOutput")

    with tile.TileContext(nc) as tc:
        _build_body(nc, tc, xT, wqkv, wo2, relT, relbc, ident, outh, dens2)
    _split_multiwaits(nc)
    return nc


def _build_body(nc, tc, xT, wqkv, wo2, relT, relbc, ident, outh, dens2):
    from contextlib import ExitStack
    ctx = ExitStack()
    consts = ctx.enter_context(tc.tile_pool(name="consts", bufs=1))
    qkv = ctx.enter_context(tc.tile_pool(name="qkv", bufs=1))
    pc = ctx.enter_context(tc.tile_pool(name="pc", bufs=2))
    skew = ctx.enter_context(tc.tile_pool(name="skew", bufs=2))
    exps = ctx.enter_context(tc.tile_pool(name="exps", bufs=6))
    outc = ctx.enter_context(tc.tile_pool(name="outc", bufs=2))
    dent = ctx.enter_context(tc.tile_pool(name="dent", bufs=2))
    psum = ctx.enter_context(tc.tile_pool(name="psum", bufs=1, space="PSUM"))
    pdram = ctx.enter_context(tc.tile_pool(name="pdram", bufs=1, space="DRAM"))

    # ---- input DMAs (constants on the Pool SWDGE queue, x on SP/ACT) ----
    # order by first use: wqkv (q proj) -> relT (P'0) -> relbc/ident (attn0)
    # -> wo (outproj, attn1)
    swqkv = consts.tile([128, 4, 3, 128], BF16, name="wqkv", tag="wqkv")
    nc.gpsimd.dma_start(
        out=swqkv[:],
        in_=wqkv.rearrange("c p k m -> p c k m"))
    srelT = consts.tile([128, W], BF16, name="relT", tag="relT")
    nc.gpsimd.dma_start(out=srelT[:], in_=relT[:, :])
    srelbc = consts.tile([128, 256], BF16, name="relbc", tag="relbc")
    nc.gpsimd.dma_start(out=srelbc[:], in_=relbc[:, :])
    sident = consts.tile([128, 128], F16, name="ident", tag="ident")
    nc.gpsimd.dma_start(out=sident[:], in_=ident[:, :])
    swo = consts.tile([128, D], F32, name="wo", tag="wo")
    nc.gpsimd.dma_start(out=_r(swo[:]), in_=_r(wo2[:, :]))
    sbias = consts.tile([128, 1], F32, name="sbias", tag="sbias")
    nc.gpsimd.memset(sbias[:], SHIFT)

    sxT = [consts.tile([128, N], BF16, name=f"xT{i}", tag=f"xT{i}")
           for i in range(4)]
    for i in range(4):
        eng = nc.sync if i % 2 == 0 else nc.scalar
        eng.dma_start(out=sxT[i][:], in_=xT[i * 128:(i + 1) * 128, :])

    # ---- SBUF working tensors ----
    qT = qkv.tile([128, N], BF16, name="qT", tag="qT")
    kT = qkv.tile([128, N], BF16, name="kT", tag="kT")
    # v with ones column per (t, head): [n-in-tile, t, h, d|1]
    vta = qkv.tile([128, NJT, 2, 65], BF16, name="vta", tag="vta")
    nc.gpsimd.memset(vta[:, :, :, 64:65], 1.0)
    # ah holds both heads' unnormalized numerators: rows 0-63 h0, 64-127 h1
    ah = qkv.tile([128, N], F32, name="ah", tag="ah")

    # PSUM: pos 2 banks + pst 2 tags x 2 banks + pp 2 banks = 8 banks
    def pst_tile(i):
        return psum.tile([128, 1024], F32, name="pst", tag=f"pst{i % 2}")

    def pp_tile():
        return psum.tile([128, 1024], F32, name="pp", tag="pp")

    # ---- per-qc PR dram tensors [2h, 512q, W] ----
    prd = {qc: pdram.tile([2, 512, W], F16, name=f"pr{qc}", tag=f"pr{qc}")
           for qc in range(NQC)}

    # ---- projection units ----
    def q_pair_unit(pair):
        ps = pst_tile(pair)
        for nl in range(2):
            nch = 2 * pair + nl
            ns = slice(nch * 512, nch * 512 + 512)
            for c in range(4):
                nc.tensor.matmul(ps[:, nl * 512:nl * 512 + 512],
                                 swqkv[:, c, 0, :], sxT[c][:, ns],
                                 start=(c == 0), stop=(c == 3))
        nc.vector.tensor_copy(out=qT[:, pair * 1024:pair * 1024 + 1024],
                              in_=ps[:])

    def k_pair_unit(pair):
        ps = pst_tile(pair)
        for nl in range(2):
            nch = 2 * pair + nl
            ns = slice(nch * 512, nch * 512 + 512)
            for c in range(4):
                nc.tensor.matmul(ps[:, nl * 512:nl * 512 + 512],
                                 swqkv[:, c, 1, :], sxT[c][:, ns],
                                 start=(c == 0), stop=(c == 3))
        nc.vector.tensor_copy(out=kT[:, pair * 1024:pair * 1024 + 1024],
                              in_=ps[:])

    def v_group_unit(g):
        # tiles t = 4g .. 4g+3 -> one [128, 512] half of a pst tile
        ps = pst_tile(g)
        half = ps[:, (g % 2) * 512:(g % 2) * 512 + 512]
        for tl in range(4):
            t = 4 * g + tl
            nst = slice(t * 128, t * 128 + 128)
            for c in range(4):
                nc.tensor.matmul(half[:, tl * 128:tl * 128 + 128],
                                 sxT[c][:, nst], swqkv[:, c, 2, :],
                                 start=(c == 0), stop=(c == 3))
        nc.vector.tensor_copy(
            out=vta[:, 4 * g:4 * g + 4, :, 0:64],
            in_=half.rearrange("p (t h d) -> p t h d", t=4, h=2))

    # ---- P' production units for q-chunk qc ----
    def p_units_for(qc):
        """Fine-grained closures producing P'(qc): one per (qt, ci) matmul+
        copy step, plus one per qt for the PR write DMA."""
        if qc >= NQC:
            return []
        units = []
        cis = LIVE_CI[qc]
        state = {}
        for qt_local in range(4):
            qt = 4 * qc + qt_local

            for k, ci in enumerate(cis):
                def do_ci(qt=qt, ci=ci, first=(k == 0)):
                    if first:
                        state[qt] = pc.tile([128, 2, W], F16, name="pct",
                                            tag="pct")
                    qs = slice(qt * 128, qt * 128 + 128)
                    cs = slice(ci * 512, ci * 512 + 512)
                    ps = pp_tile()
                    for h in range(2):
                        hs = slice(h * 64, h * 64 + 64)
                        nc.tensor.matmul(ps[:, h * 512:h * 512 + 512],
                                         qT[hs, qs], srelT[hs, cs],
                                         start=True, stop=True,
                                         tile_position=(h * 64, 0))
                    nc.vector.tensor_copy(
                        out=state[qt][:, :, cs],
                        in_=ps[:].rearrange("p (h s) -> p h s", h=2))

                units.append(do_ci)

            def write_qt(qt=qt, qt_local=qt_local):
                lo, hi = cis[0] * 512, cis[-1] * 512 + 512
                rows = slice(qt_local * 128, qt_local * 128 + 128)
                pct = state[qt]
                nc.gpsimd.dma_start(
                    out=prd[qc][:, rows, lo:hi].rearrange("h r w -> r h w"),
                    in_=pct[:, :, lo:hi])

            units.append(write_qt)
        return units

    def emit_skew_read(qc):
        """One transposing DMA per head covering all in-band j-tiles.
        Both on the SP queue: concurrent XBAR transposes on different
        queues corrupt each other (verified on HW)."""
        jt_min, njt = IN_BAND[qc]
        tiles = {}
        t = prd[qc]
        for h, eng in ((0, nc.sync), (1, nc.sync)):
            skt = skew.tile([128, 12, 512], F16, name="skt", tag=f"skt{h}")
            src = bass.AP(tensor=t.tensor,
                          offset=t.offset + h * 512 * W
                          + 128 * jt_min - 512 * qc + 1023,
                          ap=[[2047, 512], [1, 128 * njt]])
            eng.dma_start(out=skt[:, 0:njt, :], in_=src, transpose=True)
            tiles[h] = skt
        return tiles

    # ---- output projection units for q-chunk qc (unnormalized) ----
    def o_units_for(qc):
        if qc < 0:
            return []
        units = []
        for qt_local in range(4):
            qt = 4 * qc + qt_local

            def do_o(qt=qt):
                qs = slice(qt * 128, qt * 128 + 128)
                ps = pp_tile()
                for h in range(2):
                    hs = slice(h * 64, h * 64 + 64)
                    nc.tensor.matmul(ps[:, h * 512:h * 512 + 512],
                                     _r(ah[hs, qs]), _r(swo[hs, :]),
                                     start=True, stop=True,
                                     tile_position=(h * 64, 0))
                ot = outc.tile([128, 1024], F16, name="oc", tag="oc")
                nc.scalar.activation(out=ot[:], in_=ps[:], func=AF.Copy)
                nc.gpsimd.dma_start(
                    out=outh[:, qs, :].rearrange("h q d -> q h d"),
                    in_=ot[:].rearrange("p (h d) -> p h d", h=2))

            units.append(do_o)
        return units

    # ---- attention for one q-chunk ----
    def emit_attn(qc, skt, fill_units):
        jt_min, njt = IN_BAND[qc]
        in_band = lambda jt: jt_min <= jt < jt_min + njt
        jts = [jt for jt in range(NJT) if not in_band(jt)] + \
              [jt for jt in range(NJT) if in_band(jt)]
        qs = slice(qc * 512, qc * 512 + 512)
        pot = [psum.tile([65, 512], F32, name="po", tag=f"po{h}")
               for h in range(2)]
        ets = {}
        fu = iter(fill_units)
        nfill = len(fill_units)

        def emit_pv(pi):
            jt = jts[pi]
            for h in range(2):
                nc.tensor.matmul(pot[h][:], vta[:, jt, h, :],
                                 ets[jt][:, h * 512:h * 512 + 512],
                                 start=(pi == 0), stop=(pi == NJT - 1))

        done_fill = 0
        for pi, jt in enumerate(jts):
            js = slice(jt * 128, jt * 128 + 128)
            ps = pst_tile(pi)
            for h in range(2):
                hs = slice(h * 64, h * 64 + 64)
                nc.tensor.matmul(ps[:, h * 512:h * 512 + 512],
                                 kT[hs, js], qT[hs, qs],
                                 start=True, stop=False,
                                 tile_position=(h * 64, 0))
            if in_band(jt):
                for h in range(2):
                    nc.tensor.matmul(ps[:, h * 512:h * 512 + 512],
                                     sident[:], skt[h][:, jt - jt_min, :],
                                     start=False, stop=True)
            else:
                A = qc * 512 + 512 - 128 * jt
                bc = 0 if A <= -512 else 128
                for h in range(2):
                    hs = slice(h * 64, h * 64 + 64)
                    nc.tensor.matmul(ps[:, h * 512:h * 512 + 512],
                                     srelbc[hs, bc:bc + 128], qT[hs, qs],
                                     start=False, stop=True,
                                     tile_position=(h * 64, 0))
            et = exps.tile([128, 1024], BF16, name="expS", tag="expS")
            nc.scalar.activation(out=et[:], in_=ps[:], func=AF.Exp,
                                 bias=sbias[:])
            ets[jt] = et
            if pi > 0:
                emit_pv(pi - 1)
            # front-load fill units: finish them by step 12 so the next
            # skew DMA (last P'-chain unit) has time to land
            want = min(nfill, (nfill * (pi + 1) + 11) // 12)
            while done_fill < want:
                u = next(fu, None)
                if u is None:
                    break
                u()
                done_fill += 1
        emit_pv(NJT - 1)
        for u in fu:
            u()
        # numerators -> ah (f32), denominators -> dram (f16)
        dt = dent.tile([1, 2, 512], F16, name="den", tag="den")
        for h in range(2):
            hs = slice(h * 64, h * 64 + 64)
            nc.vector.tensor_copy(out=_r(ah[hs, qs]), in_=_r(pot[h][0:64, :]))
            nc.vector.tensor_copy(out=dt[:, h, :], in_=pot[h][64:65, :])
        nc.gpsimd.dma_start(out=dens2[qc:qc + 1, :, :], in_=dt[:])

    # ---- prologue: x DMAs || q proj || P'(0) || k/v proj ----
    p0 = p_units_for(0)  # 3 ci-units + 1 write per qt -> 16 units
    q_pair_unit(0)
    q_pair_unit(1)
    # interleave: P'(0) units with k and v projection (PE filler while the
    # DVE casts and PR-write DMAs drain)
    p0i = iter(p0)
    next(p0i)()                     # qt0 ci0
    k_pair_unit(0)
    next(p0i)(); next(p0i)()        # qt0 ci1, ci2
    k_pair_unit(1)
    next(p0i)()                     # qt0 write
    v_group_unit(0)
    next(p0i)(); next(p0i)()        # qt1 ci0, ci1
    v_group_unit(1)
    next(p0i)(); next(p0i)()        # qt1 ci2, write
    v_group_unit(2)
    next(p0i)(); next(p0i)()        # qt2 ci0, ci1
    v_group_unit(3)
    for u in p0i:                   # qt2 ci2+write, qt3 all
        u()
    skt = emit_skew_read(0)

    # ---- main loop: attn(qc) with P'(qc+1) + skew(qc+1) + outproj(qc-1) ----
    for qc in range(NQC):
        fills = p_units_for(qc + 1)
        if qc + 1 < NQC:
            fills.append(lambda qc=qc: skew_next.update(emit_skew_read(qc + 1)))
        fills += o_units_for(qc - 1)
        skew_next = {}
        emit_attn(qc, skt, fills)
        if qc + 1 < NQC:
            skt = dict(skew_next)

    # tail: outproj for the last q-chunk
    for u in o_units_for(NQC - 1):
        u()
    ctx.close()


_NC_CACHE = [None]


def _get_nc():
    if _NC_CACHE[0] is None:
        _NC_CACHE[0] = build_kernel()
    return _NC_CACHE[0]


def make_in_maps(x, Wq, Wkv, Wo, bo, rel_emb):
    bf16 = ml_dtypes.bfloat16
    xT = [np.ascontiguousarray(x[b].T).astype(bf16) for b in range(2)]
    cols = np.arange(W)
    idx = np.clip(1535 - cols, 0, 1024)
    relT = np.empty((128, W), np.float32)
    relT[0:64] = rel_emb[idx].T
    relT[64:128] = relT[0:64]
    relT = relT.astype(bf16)                       # reversed rel table
    relbc = np.empty((128, 256), np.float32)
    relbc[0:64, 0:128] = rel_emb[0][:, None]       # clamp-low value
    relbc[0:64, 128:256] = rel_emb[1024][:, None]  # clamp-high value
    relbc[64:128] = relbc[0:64]
    relbc = relbc.astype(bf16)
    ident = np.eye(128, dtype=np.float16)
    in_maps = []
    for c in range(8):
        b, hp = c // 4, c % 4
        cs = slice(hp * 128, hp * 128 + 128)
        wqkv = np.empty((4, 128, 3, 128), np.float32)
        for ci in range(4):
            rows = slice(ci * 128, ci * 128 + 128)
            wqkv[ci, :, 0, :] = Wq[rows, cs] / 8.0
            wqkv[ci, :, 1, :] = Wkv[rows, :512][:, cs]
            wqkv[ci, :, 2, :] = Wkv[rows, 512:][:, cs]
        in_maps.append({
            "xT": xT[b],
            "wqkv": wqkv.astype(bf16),
            "wo2": np.ascontiguousarray(Wo[cs, :]).astype(np.float32),
            "relT": relT,
            "relbc": relbc,
            "ident": ident,
        })
    return in_maps


def run(x, Wq, Wkv, Wo, bo, rel_emb, trace=False, trace_cores=None):
    nc = _get_nc()
    in_maps = make_in_maps(x, Wq, Wkv, Wo, bo, rel_emb)
    res = run_bass_kernel_spmd(nc, in_maps, core_ids=list(range(8)),
                               trace=trace, trace_cores=trace_cores)
    out = np.zeros((2, N, D), np.float32)
    for c in range(8):
        b = c // 4
        num = np.asarray(res.results[c]["outh"], np.float32)   # [2, N, D]
        den = np.asarray(res.results[c]["dens2"], np.float32)  # [4, 2, 512]
        for h in range(2):
            out[b] += num[h] / den[:, h, :].reshape(N)[:, None]
    out += np.asarray(bo, np.float32)[None, None, :]
    return out, res


def kernel(x, Wq, Wkv, Wo, bo, rel_emb):
    out, _ = run(np.asarray(x), np.asarray(Wq), np.asarray(Wkv),
                 np.asarray(Wo), np.asarray(bo), np.asarray(rel_emb))
    return out
